# revision 9
# baseline (speedup 1.0000x reference)
"""APoT quantizer (vq_codebook) distributed Bass kernel for 8 TRN2 NeuronCores.

Sharding: data-parallel along dim 0 (4096 rows -> 512 rows/core); alpha-derived
scalars are replicated per-partition via a tiny consts tensor.

Algorithm (signed-mantissa bit tricks): the APoT level set for bits=8, k=2,
signed is exactly the two-hot set {0} U {+-(2^-p + 2^-q)}, so nearest-level
quantization decomposes into float bit ops -- no table search.

Fast variant ("h16") runs the bit tricks in fp16 so the DVE ops hit the
16-bit 2x/4x perf modes (this kernel is HBM-bound; the goal is keeping both
compute engines well under the DMA roofline):

  z    = fp16(x * (1/alpha))               [ACT Copy f32->fp16, AP scale]
  b    = z & 0xFC00                        [sign+exponent = signed po2 s*2^E]
  m    = (z & 0x03FF) | 0x3C00             [mantissa in [1,2)]
  r'   = (m - 1)*C2H                       [fp16 TS (sub,mult); C2H=0x3D55,
                                            largest fp16 < 4/3 => exponent
                                            field of r' is round-to-nearest-
                                            po2 of (m-1); exact ties round
                                            up (measure-zero vs reference)]
  rq   = r' & 0x7C00
  yq   = (rq + 1) * b                      [DVE stt, fp16 in / f32 out; DVE
                                            ALU is f32 internally => exact]
  out  = yq * alpha                        [ACT Copy, full f32 alpha]

fp16 z-rounding (11-bit) can move elements across quantization boundaries
(out by one level) and drops the second term for levels with j > 10; both
effects are ~2^-11-relative. Norm rel-err on the N(0,1)/alpha=max|x| data:
~2e-3 (tolerance 2e-2). Tiny |z| < 2^-14 flushes to 0 -> out 0 (matches
the reference's zero level within ~2^-14*alpha).

Variants: "h16" needs max|x| <= alpha (true for the reference setup where
alpha = |x|.max()); "h16_safe" adds a fp16 clamp; "gen" is the full-f32
7-op fallback for any alpha. kernel() picks on the host.
"""
import os
import numpy as np
from contextlib import ExitStack

N_CORES = 8
ROWS, COLS = 4096, 16384
SHARD_ROWS = ROWS // N_CORES  # 512
P = 128
PB = SHARD_ROWS // P          # partition blocks per core

EPS = 1e-8
C2 = float(np.uint32(0x3FAAAAAA).view(np.float32))          # f32 just below 4/3
CLAMP_HI = float(np.uint32(0x3F7FFFFF).view(np.float32))    # 1 - 2^-24
C2H = 1.3330078125            # fp16 0x3D55, largest fp16 < 4/3
CLAMP_HI_H = 0.99951171875    # fp16 0x3BFF, largest fp16 < 1

FD = int(os.environ.get("KV_FD", "4096"))
IO_BUFS = int(os.environ.get("KV_IO_BUFS", "3"))
O_BUFS = int(os.environ.get("KV_O_BUFS", "3"))
MID_BUFS = int(os.environ.get("KV_MID_BUFS", "3"))
OUT_ENG = os.environ.get("KV_OUT_ENG", "sync")

_CACHE = {}


def _build(variant="h16", fd=FD, io_bufs=IO_BUFS, o_bufs=O_BUFS,
           mid_bufs=MID_BUFS, out_eng=OUT_ENG):
    import concourse.tile as tile
    from concourse import bacc, mybir

    F32 = mybir.dt.float32
    U32 = mybir.dt.uint32
    F16 = mybir.dt.float16
    U16 = mybir.dt.uint16
    ALU = mybir.AluOpType
    ACTF = mybir.ActivationFunctionType

    nc = bacc.Bacc("TRN2", target_bir_lowering=False, debug=False,
                   num_devices=N_CORES)
    x_d = nc.dram_tensor("x", [SHARD_ROWS, COLS], F32, kind="ExternalInput")
    c_d = nc.dram_tensor("consts", [P, 8], U32, kind="ExternalInput")
    o_d = nc.dram_tensor("out", [SHARD_ROWS, COLS], F32, kind="ExternalOutput")

    nt = COLS // fd
    with tile.TileContext(nc) as tc, ExitStack() as ctx:
        cpool = ctx.enter_context(tc.tile_pool(name="cp", bufs=1))
        xio = ctx.enter_context(tc.tile_pool(name="xio", bufs=io_bufs))
        zp = ctx.enter_context(tc.tile_pool(name="zp", bufs=mid_bufs))
        bp = ctx.enter_context(tc.tile_pool(name="bp", bufs=mid_bufs))
        mp = ctx.enter_context(tc.tile_pool(name="mp", bufs=mid_bufs))
        op = ctx.enter_context(tc.tile_pool(name="op", bufs=o_bufs))

        consts = cpool.tile([P, 8], U32)
        nc.sync.dma_start(consts[:], c_d[:])
        ia_ap = consts[:, 1:2].bitcast(F32)        # 1/a
        a_ap = consts[:, 2:3].bitcast(F32)         # a
        out_dma = nc.scalar if out_eng == "scalar" else nc.sync

        for blk in range(PB):
            rows = slice(blk * P, (blk + 1) * P)
            for j in range(nt):
                cols = slice(j * fd, (j + 1) * fd)
                t_x = xio.tile([P, fd], F32, tag="t_x")
                nc.sync.dma_start(t_x[:], x_d[rows, cols])

                t_o = op.tile([P, fd], F32, tag="t_o")
                if variant in ("h16", "h16_safe"):
                    t_z = zp.tile([P, fd], F16, tag="t_z")
                    t_b = bp.tile([P, fd], F16, tag="t_b")
                    t_m = mp.tile([P, fd], F16, tag="t_m")
                    # z = fp16(x * inv_a)   [ACT]
                    nc.scalar.activation(t_z[:], t_x[:], ACTF.Copy,
                                         bias=0.0, scale=ia_ap)
                    if variant == "h16_safe":
                        nc.vector.tensor_scalar(t_z[:], t_z[:],
                                                CLAMP_HI_H, -CLAMP_HI_H,
                                                ALU.min, ALU.max)
                    # b = z & 0xFC00   [DVE]
                    nc.vector.tensor_scalar(t_b[:].bitcast(U16),
                                            t_z[:].bitcast(U16),
                                            0xFC00, None, ALU.bitwise_and)
                    # m = (z & 0x03FF) | 0x3C00   [DVE]
                    nc.vector.tensor_scalar(t_m[:].bitcast(U16),
                                            t_z[:].bitcast(U16),
                                            0x03FF, 0x3C00,
                                            ALU.bitwise_and, ALU.bitwise_or)
                    # r' = (m - 1) * C2H   [DVE, in place]
                    nc.vector.tensor_scalar(t_m[:], t_m[:], 1.0, C2H,
                                            ALU.subtract, ALU.mult)
                    # rq = r' & 0x7C00   [DVE, in place]
                    nc.vector.tensor_scalar(t_m[:].bitcast(U16),
                                            t_m[:].bitcast(U16),
                                            0x7C00, None, ALU.bitwise_and)
                    # yq = (rq + 1) * b   [DVE stt, all-fp16 2x mode, in place]
                    nc.vector.scalar_tensor_tensor(
                        t_b[:], t_m[:], 1.0, t_b[:], ALU.add, ALU.mult)
                    # out = yq * a   [ACT, fp16 -> f32 upcast + scale]
                    nc.scalar.activation(t_o[:], t_b[:], ACTF.Copy,
                                         bias=0.0, scale=a_ap)
                else:  # "gen": full-f32 reference path, any alpha
                    t_z = zp.tile([P, fd], F32, tag="t_z")
                    t_b = bp.tile([P, fd], F32, tag="t_b")
                    t_m = mp.tile([P, fd], F32, tag="t_m")
                    nc.scalar.activation(t_z[:], t_x[:], ACTF.Copy,
                                         bias=0.0, scale=ia_ap)
                    nc.vector.tensor_scalar(t_z[:], t_z[:],
                                            CLAMP_HI, -CLAMP_HI,
                                            ALU.min, ALU.max)
                    nc.vector.tensor_scalar(t_b[:].bitcast(U32),
                                            t_z[:].bitcast(U32),
                                            0xFF800000, None, ALU.bitwise_and)
                    nc.vector.tensor_scalar(t_m[:].bitcast(U32),
                                            t_z[:].bitcast(U32),
                                            0x007FFFFF, 0x3F800000,
                                            ALU.bitwise_and, ALU.bitwise_or)
                    nc.scalar.activation(t_m[:], t_m[:], ACTF.Copy,
                                         bias=-C2, scale=C2)
                    nc.vector.tensor_scalar(t_m[:].bitcast(U32),
                                            t_m[:].bitcast(U32),
                                            0x7F800000, None, ALU.bitwise_and)
                    nc.vector.scalar_tensor_tensor(
                        t_o[:], t_m[:], 1.0, t_b[:], ALU.add, ALU.mult)
                    nc.scalar.activation(t_o[:], t_o[:], ACTF.Copy,
                                         bias=0.0, scale=a_ap)

                out_dma.dma_start(o_d[rows, cols], t_o[:])
    nc.compile()
    return nc


def _get_nc(variant, **kw):
    key = (variant, tuple(sorted(kw.items())))
    if key not in _CACHE:
        _CACHE[key] = _build(variant, **kw)
    return _CACHE[key]


def _pick_variant(x, alpha):
    a = np.float32(max(float(np.asarray(alpha, dtype=np.float32)), EPS))
    xmax = float(np.abs(x).max())
    if float(a) >= 6.2e-5:  # fp16 min normal ~6.1e-5; below that stay f32
        return "h16" if xmax <= float(a) else "h16_safe"
    return "gen"


def make_consts(alpha):
    a = np.float32(max(float(np.asarray(alpha, dtype=np.float32)), EPS))
    inv_a = np.float32(1.0) / a
    consts = np.zeros((P, 8), np.uint32)
    consts[:, 1] = inv_a.view(np.uint32)
    consts[:, 2] = a.view(np.uint32)
    return consts


def make_in_maps(inputs):
    x = np.ascontiguousarray(np.asarray(inputs["x"], dtype=np.float32))
    consts = make_consts(inputs["alpha"])
    return [
        {"x": x[i * SHARD_ROWS:(i + 1) * SHARD_ROWS], "consts": consts}
        for i in range(N_CORES)
    ]


def get_nc_for(inputs):
    x = np.asarray(inputs["x"], dtype=np.float32)
    return _get_nc(_pick_variant(x, inputs["alpha"]))


def kernel(x, alpha, levels=None):
    """Full-input entry point. x: [4096,16384] f32, alpha: scalar f32."""
    from concourse.bass_utils import run_bass_kernel_spmd

    x = np.ascontiguousarray(np.asarray(x, dtype=np.float32))
    nc = _get_nc(_pick_variant(x, alpha))
    consts = make_consts(alpha)
    in_maps = [
        {"x": x[i * SHARD_ROWS:(i + 1) * SHARD_ROWS], "consts": consts}
        for i in range(N_CORES)
    ]
    res = run_bass_kernel_spmd(nc, in_maps, core_ids=list(range(N_CORES)))
    out = np.concatenate([res.results[i]["out"] for i in range(N_CORES)],
                         axis=0)
    return out.astype(np.float32)


# revision 11
# speedup vs baseline: 1.2169x; 1.2169x over previous
"""APoT quantizer (vq_codebook) distributed Bass kernel for 8 TRN2 NeuronCores.

Sharding: data-parallel along dim 0 (4096 rows -> 512 rows/core); alpha-derived
scalars are replicated per-partition via a tiny consts tensor.

Algorithm (signed-mantissa bit tricks): the APoT level set for bits=8, k=2,
signed is exactly the two-hot set {0} U {+-(2^-p + 2^-q)}, so nearest-level
quantization decomposes into float bit ops -- no table search.

Fast variant ("h16") runs the bit tricks in fp16 so the DVE ops hit the
16-bit 2x/4x perf modes (this kernel is HBM-bound; the goal is keeping both
compute engines well under the DMA roofline):

  z    = fp16(x * (1/alpha))               [ACT Copy f32->fp16, AP scale]
  b    = z & 0xFC00                        [sign+exponent = signed po2 s*2^E]
  m    = (z & 0x03FF) | 0x3C00             [mantissa in [1,2)]
  r'   = (m - 1)*C2H                       [fp16 TS (sub,mult); C2H=0x3D55,
                                            largest fp16 < 4/3 => exponent
                                            field of r' is round-to-nearest-
                                            po2 of (m-1); exact ties round
                                            up (measure-zero vs reference)]
  rq   = r' & 0x7C00
  yq   = (rq + 1) * b                      [DVE stt, fp16 in / f32 out; DVE
                                            ALU is f32 internally => exact]
  out  = yq * alpha                        [ACT Copy, full f32 alpha]

fp16 z-rounding (11-bit) can move elements across quantization boundaries
(out by one level) and drops the second term for levels with j > 10; both
effects are ~2^-11-relative. Norm rel-err on the N(0,1)/alpha=max|x| data:
~2e-3 (tolerance 2e-2). Tiny |z| < 2^-14 flushes to 0 -> out 0 (matches
the reference's zero level within ~2^-14*alpha).

Variants: "h16" needs max|x| <= alpha (true for the reference setup where
alpha = |x|.max()); "h16_safe" adds a fp16 clamp; "gen" is the full-f32
7-op fallback for any alpha. kernel() picks on the host.
"""
import os
import numpy as np
from contextlib import ExitStack

N_CORES = 8
ROWS, COLS = 4096, 16384
SHARD_ROWS = ROWS // N_CORES  # 512
P = 128
PB = SHARD_ROWS // P          # partition blocks per core

EPS = 1e-8
C2 = float(np.uint32(0x3FAAAAAA).view(np.float32))          # f32 just below 4/3
CLAMP_HI = float(np.uint32(0x3F7FFFFF).view(np.float32))    # 1 - 2^-24
C2H = 1.3330078125            # fp16 0x3D55, largest fp16 < 4/3
CLAMP_HI_H = 0.99951171875    # fp16 0x3BFF, largest fp16 < 1

FD = int(os.environ.get("KV_FD", "4096"))
IO_BUFS = int(os.environ.get("KV_IO_BUFS", "4"))
O_BUFS = int(os.environ.get("KV_O_BUFS", "3"))
MID_BUFS = int(os.environ.get("KV_MID_BUFS", "3"))
OUT_ENG = os.environ.get("KV_OUT_ENG", "scalar")

_CACHE = {}


def _build(variant="h16", fd=FD, io_bufs=IO_BUFS, o_bufs=O_BUFS,
           mid_bufs=MID_BUFS, out_eng=OUT_ENG):
    import concourse.tile as tile
    from concourse import bacc, mybir

    F32 = mybir.dt.float32
    U32 = mybir.dt.uint32
    F16 = mybir.dt.float16
    U16 = mybir.dt.uint16
    ALU = mybir.AluOpType
    ACTF = mybir.ActivationFunctionType

    nc = bacc.Bacc("TRN2", target_bir_lowering=False, debug=False,
                   num_devices=N_CORES)
    x_d = nc.dram_tensor("x", [SHARD_ROWS, COLS], F32, kind="ExternalInput")
    c_d = nc.dram_tensor("consts", [P, 8], U32, kind="ExternalInput")
    o_d = nc.dram_tensor("out", [SHARD_ROWS, COLS], F32, kind="ExternalOutput")

    nt = COLS // fd
    with tile.TileContext(nc) as tc, ExitStack() as ctx:
        cpool = ctx.enter_context(tc.tile_pool(name="cp", bufs=1))
        xio = ctx.enter_context(tc.tile_pool(name="xio", bufs=io_bufs))
        zp = ctx.enter_context(tc.tile_pool(name="zp", bufs=mid_bufs))
        bp = ctx.enter_context(tc.tile_pool(name="bp", bufs=mid_bufs))
        mp = ctx.enter_context(tc.tile_pool(name="mp", bufs=mid_bufs))
        op = ctx.enter_context(tc.tile_pool(name="op", bufs=o_bufs))

        consts = cpool.tile([P, 8], U32)
        nc.sync.dma_start(consts[:], c_d[:])
        ia_ap = consts[:, 1:2].bitcast(F32)        # 1/a
        a_ap = consts[:, 2:3].bitcast(F32)         # a
        out_dma = nc.scalar if out_eng == "scalar" else nc.sync

        ntiles = PB * nt
        SKEW = 2

        def tile_rc(i):
            blk, j = divmod(i, nt)
            return (slice(blk * P, (blk + 1) * P),
                    slice((i % nt) * fd, ((i % nt) + 1) * fd))

        if variant in ("h16", "h16_safe"):
            # Software-pipelined: ACT's z-pass leads its scale-pass by SKEW
            # tiles so neither engine head-of-line blocks the other; the
            # out-DMA issues from the ACT HWDGE ring right after the scale
            # op (same engine => program order, no semaphore round trip).
            live = {}
            for it in range(ntiles + SKEW):
                if it < ntiles:
                    rows, cols = tile_rc(it)
                    t_x = xio.tile([P, fd], F32, tag="t_x")
                    nc.sync.dma_start(t_x[:], x_d[rows, cols])
                    t_z = zp.tile([P, fd], F16, tag="t_z")
                    t_b = bp.tile([P, fd], F16, tag="t_b")
                    t_m = mp.tile([P, fd], F16, tag="t_m")
                    live[it] = (t_b,)
                    # z = fp16(x * inv_a)   [ACT]
                    nc.scalar.activation(t_z[:], t_x[:], ACTF.Copy,
                                         bias=0.0, scale=ia_ap)
                    if variant == "h16_safe":
                        nc.vector.tensor_scalar(t_z[:], t_z[:],
                                                CLAMP_HI_H, -CLAMP_HI_H,
                                                ALU.min, ALU.max)
                    # b = z & 0xFC00   [DVE 4x]
                    nc.vector.tensor_scalar(t_b[:].bitcast(U16),
                                            t_z[:].bitcast(U16),
                                            0xFC00, None, ALU.bitwise_and)
                    # m = (z & 0x03FF) | 0x3C00   [DVE 4x]
                    nc.vector.tensor_scalar(t_m[:].bitcast(U16),
                                            t_z[:].bitcast(U16),
                                            0x03FF, 0x3C00,
                                            ALU.bitwise_and, ALU.bitwise_or)
                    # r' = (m - 1) * C2H   [DVE 4x, in place]
                    nc.vector.tensor_scalar(t_m[:], t_m[:], 1.0, C2H,
                                            ALU.subtract, ALU.mult)
                    # mq = (r' & 0x7C00) ... + 1 below   [DVE 4x, in place]
                    nc.vector.tensor_scalar(t_m[:].bitcast(U16),
                                            t_m[:].bitcast(U16),
                                            0x7C00, None, ALU.bitwise_and)
                    nc.vector.tensor_scalar(t_m[:], t_m[:], 1.0, None,
                                            ALU.add)
                    # yq = mq * b   [DVE tensor_tensor fp16 2x, in place]
                    nc.vector.tensor_tensor(t_b[:], t_m[:], t_b[:], ALU.mult)
                if it >= SKEW:
                    i = it - SKEW
                    rows, cols = tile_rc(i)
                    (t_b,) = live.pop(i)
                    t_o = op.tile([P, fd], F32, tag="t_o")
                    # out = yq * a   [ACT, fp16 -> f32 upcast + scale]
                    nc.scalar.activation(t_o[:], t_b[:], ACTF.Copy,
                                         bias=0.0, scale=a_ap)
                    out_dma.dma_start(o_d[rows, cols], t_o[:])
        else:  # "gen": full-f32 reference path, any alpha
            for i in range(ntiles):
                rows, cols = tile_rc(i)
                t_x = xio.tile([P, fd], F32, tag="t_x")
                nc.sync.dma_start(t_x[:], x_d[rows, cols])
                t_o = op.tile([P, fd], F32, tag="t_o")
                t_z = zp.tile([P, fd], F32, tag="t_z")
                t_b = bp.tile([P, fd], F32, tag="t_b")
                t_m = mp.tile([P, fd], F32, tag="t_m")
                nc.scalar.activation(t_z[:], t_x[:], ACTF.Copy,
                                     bias=0.0, scale=ia_ap)
                nc.vector.tensor_scalar(t_z[:], t_z[:],
                                        CLAMP_HI, -CLAMP_HI,
                                        ALU.min, ALU.max)
                nc.vector.tensor_scalar(t_b[:].bitcast(U32),
                                        t_z[:].bitcast(U32),
                                        0xFF800000, None, ALU.bitwise_and)
                nc.vector.tensor_scalar(t_m[:].bitcast(U32),
                                        t_z[:].bitcast(U32),
                                        0x007FFFFF, 0x3F800000,
                                        ALU.bitwise_and, ALU.bitwise_or)
                nc.scalar.activation(t_m[:], t_m[:], ACTF.Copy,
                                     bias=-C2, scale=C2)
                nc.vector.tensor_scalar(t_m[:].bitcast(U32),
                                        t_m[:].bitcast(U32),
                                        0x7F800000, None, ALU.bitwise_and)
                nc.vector.scalar_tensor_tensor(
                    t_o[:], t_m[:], 1.0, t_b[:], ALU.add, ALU.mult)
                nc.scalar.activation(t_o[:], t_o[:], ACTF.Copy,
                                     bias=0.0, scale=a_ap)
                out_dma.dma_start(o_d[rows, cols], t_o[:])
    nc.compile()
    return nc


def _get_nc(variant, **kw):
    key = (variant, tuple(sorted(kw.items())))
    if key not in _CACHE:
        _CACHE[key] = _build(variant, **kw)
    return _CACHE[key]


def _pick_variant(x, alpha):
    a = np.float32(max(float(np.asarray(alpha, dtype=np.float32)), EPS))
    xmax = float(np.abs(x).max())
    if float(a) >= 6.2e-5:  # fp16 min normal ~6.1e-5; below that stay f32
        return "h16" if xmax <= float(a) else "h16_safe"
    return "gen"


def make_consts(alpha):
    a = np.float32(max(float(np.asarray(alpha, dtype=np.float32)), EPS))
    inv_a = np.float32(1.0) / a
    consts = np.zeros((P, 8), np.uint32)
    consts[:, 1] = inv_a.view(np.uint32)
    consts[:, 2] = a.view(np.uint32)
    return consts


def make_in_maps(inputs):
    x = np.ascontiguousarray(np.asarray(inputs["x"], dtype=np.float32))
    consts = make_consts(inputs["alpha"])
    return [
        {"x": x[i * SHARD_ROWS:(i + 1) * SHARD_ROWS], "consts": consts}
        for i in range(N_CORES)
    ]


def get_nc_for(inputs):
    x = np.asarray(inputs["x"], dtype=np.float32)
    return _get_nc(_pick_variant(x, inputs["alpha"]))


def kernel(x, alpha, levels=None):
    """Full-input entry point. x: [4096,16384] f32, alpha: scalar f32."""
    from concourse.bass_utils import run_bass_kernel_spmd

    x = np.ascontiguousarray(np.asarray(x, dtype=np.float32))
    nc = _get_nc(_pick_variant(x, alpha))
    consts = make_consts(alpha)
    in_maps = [
        {"x": x[i * SHARD_ROWS:(i + 1) * SHARD_ROWS], "consts": consts}
        for i in range(N_CORES)
    ]
    res = run_bass_kernel_spmd(nc, in_maps, core_ids=list(range(N_CORES)))
    out = np.concatenate([res.results[i]["out"] for i in range(N_CORES)],
                         axis=0)
    return out.astype(np.float32)
